# revision 1
# baseline (speedup 1.0000x reference)
"""Adstock transform on 8 trn2 cores — native DVE/GpSimd scan, bf16 HBM I/O.

r[b, t, c] = x[b, t, c] + d[c] * r[b, t-1, c]

Host-side sharding: batch split across 8 cores; each core's slab is provided
as [b_loc, C, T] bf16 (host transposes + downcasts; upcasts y back).  The 2e-2
relative tolerance dwarfs bf16 quantization (~4.5e-3 measured end-to-end).

Device: for each (batch, 2048-step block): load [C=128 partitions, 2048] bf16
(4 KiB contiguous per partition), run tensor_tensor_scan
(state = d*state + x, fp32 internal state, per-partition recurrence along the
free dim), store bf16.  Blocks chain via initial=prev[:, -1:].  Scans run on
DVE (GpSimd lacks the scan opcode in codegen); loads on sync ring, stores on
scalar ring.  HBM traffic is 16+16 MiB per core — half of fp32 — and the only
compute is the scan itself.
"""

import numpy as np
import ml_dtypes

import concourse.bacc as bacc
import concourse.mybir as mybir
from concourse.bass_utils import run_bass_kernel_spmd
from concourse.tile import TileContext

F32 = mybir.dt.float32
BF16 = mybir.dt.bfloat16
_BF16_NP = ml_dtypes.bfloat16

B, T, C = 64, 8192, 128
NCORES = 8
B_LOC = B // NCORES     # 8 batches per core
NBLK = 2048             # time steps per scan block (4 KiB bf16 per partition)
NB = T // NBLK          # 4 blocks per batch


def build_nc(b_loc=B_LOC, t_total=T, nrepeat=1):
    nblk = t_total // NBLK

    nc = bacc.Bacc("TRN2", target_bir_lowering=False, debug=False)
    x = nc.dram_tensor("x", [b_loc, C, t_total], BF16, kind="ExternalInput").ap()
    dcol = nc.dram_tensor("dcol", [C, 1], F32, kind="ExternalInput").ap()
    y = nc.dram_tensor("y", [b_loc, C, t_total], BF16, kind="ExternalOutput").ap()

    with TileContext(nc) as tc:
        with (
            tc.tile_pool(name="const", bufs=1) as cpool,
            tc.tile_pool(name="load", bufs=8) as lpool,
            tc.tile_pool(name="out", bufs=16) as spool,
        ):
            d_t = cpool.tile([C, 1], F32)
            nc.scalar.dma_start(out=d_t, in_=dcol)
            d_bc = d_t.broadcast_to([C, NBLK])

            for rep in range(nrepeat):
                prev = [None] * b_loc
                for blk in range(nblk):
                    t0 = blk * NBLK
                    for b in range(b_loc):
                        lt = lpool.tile([C, NBLK], BF16, tag="in")
                        nc.sync.dma_start(out=lt, in_=x[b, :, t0 : t0 + NBLK])
                        ot = spool.tile([C, NBLK], BF16, tag="out")
                        eng = nc.vector
                        init = (
                            0.0
                            if prev[b] is None
                            else prev[b][:, NBLK - 1 : NBLK]
                        )
                        eng.tensor_tensor_scan(
                            out=ot,
                            data0=d_bc,
                            data1=lt,
                            initial=init,
                            op0=mybir.AluOpType.mult,
                            op1=mybir.AluOpType.add,
                        )
                        nc.scalar.dma_start(out=y[b, :, t0 : t0 + NBLK], in_=ot)
                        prev[b] = ot
                # reset chains between reps (each rep recomputes from scratch)
                prev = [None] * b_loc
    nc.finalize()
    return nc


_NC_CACHE = {}


def _get_nc(nrepeat=1):
    key = (B_LOC, T, nrepeat)
    if key not in _NC_CACHE:
        _NC_CACHE[key] = build_nc(nrepeat=nrepeat)
    return _NC_CACHE[key]


def _make_consts(decay: np.ndarray):
    d = 1.0 / (1.0 + np.exp(-decay.astype(np.float64)))  # [C]
    dcol = d.astype(np.float32)[:, None].copy()
    return dcol


def make_in_maps(x, decay):
    x = np.asarray(x, dtype=np.float32)
    dcol = _make_consts(np.asarray(decay))
    return [
        {
            "x": x[i * B_LOC : (i + 1) * B_LOC]
            .transpose(0, 2, 1)
            .astype(_BF16_NP),
            "dcol": dcol,
        }
        for i in range(NCORES)
    ]


def assemble_output(y_stack):
    """[ncores, b_loc, C, T] (bf16) -> [B, T, C] fp32"""
    return np.concatenate(
        [
            np.asarray(yc).astype(np.float32).transpose(0, 2, 1)
            for yc in y_stack
        ],
        axis=0,
    )


def run(x, decay, trace=False, tmpdir=None, trace_cores=None):
    nc = _get_nc()
    in_maps = make_in_maps(x, decay)
    res = run_bass_kernel_spmd(
        nc,
        in_maps,
        list(range(NCORES)),
        trace=trace,
        tmpdir=tmpdir,
        trace_cores=trace_cores,
    )
    out = np.concatenate(
        [
            r["y"].astype(np.float32).transpose(0, 2, 1)
            for r in res.results
        ],
        axis=0,
    )
    return out, res


def kernel(x: np.ndarray, decay: np.ndarray) -> np.ndarray:
    out, _ = run(x, decay)
    return out

